# revision 53
# baseline (speedup 1.0000x reference)
"""Trainium2 Bass kernel for causal MHA (B=2, T=2048, D=1024, H=16, KH=64).

Sharding: 8 cores = 2 (batch) x 4 (head groups of 4 heads).
Each core computes q/k/v projections for its 4 heads, causal attention,
and a partial output projection against its 256-row slice of Wout.
Host sums the 4 partials per batch (the all-reduce step, done at unshard).

Single fused pipeline tuned around two facts measured in the trace:
(1) the Scalar engine's softmax EXP (~1ns/elem) is the steady-state
bottleneck of the attention inner loop, and (2) the PE p-state clock only
ramps past 2.0 GHz when the matmul queue never starves.  So projections
for t-block tb+1, the V projection, and the output projection are all
emitted as fine-grained "fill" matmuls interleaved between the score /
PV matmul pairs of the attention units, keeping the PE dense while EXPs
stream.  Normalization runs off the PE entirely: the PV stationary
carries a ones column so the softmax denominator accumulates on PSUM
partition 64, a 1-lane DVE fast-reciprocal pulls 1/S into SBUF, GPSIMD
partition-broadcasts it, and DVE applies it.  Score matmul pairs use the
PE's 64x128 row tiles (head-even rows 0:63, head-odd rows 64:127); a
post-compile pass hoists the second tile's LDWEIGHTS above the first
tile's MATMUL so the two row tiles execute concurrently.
"""
import os
import sys

sys.path.insert(0, "/opt/trn_rl_repo")

from collections import deque
from contextlib import ExitStack

import numpy as np

import concourse.bacc as bacc
import concourse.mybir as mybir
import concourse.tile as tile

B, T, C = 2, 2048, 1024
H, KH = 16, 64
G = 4                 # head groups
HPG = H // G          # heads per group = 4
DG = HPG * KH         # 256 per-core head dims
NCORES = 8
NK = C // 128         # 8 contraction chunks
NT = T // 512         # 4 t blocks
NTT = T // 128        # 16 t tiles

F32 = mybir.dt.float32
BF16 = mybir.dt.bfloat16
EXP = mybir.ActivationFunctionType.Exp

HOIST = os.environ.get("KERNEL_HOIST", "1") == "1"

_cached_nc = None


def build_nc():
    nc = bacc.Bacc()
    xt = nc.dram_tensor("xt", [NT * C, 512], BF16, kind="ExternalInput")  # x[b].T by t-block
    wq = nc.dram_tensor("wq", [C, DG], BF16, kind="ExternalInput")       # Wq slice .T
    wk = nc.dram_tensor("wk", [C, DG], BF16, kind="ExternalInput")
    wv = nc.dram_tensor("wv", [C, DG], BF16, kind="ExternalInput")
    wo = nc.dram_tensor("wo", [DG, C], BF16, kind="ExternalInput")       # Wout[:, slice].T
    keep = nc.dram_tensor("keep", [128, T], BF16, kind="ExternalInput")  # diag keep blocks (k, q)
    y = nc.dram_tensor("y", [T, C], BF16, kind="ExternalOutput")         # partial output

    with ExitStack() as ctx:
        ctx.enter_context(nc.allow_low_precision(reason="bf16 matmul pipeline"))
        tc = ctx.enter_context(tile.TileContext(nc))
        persist = ctx.enter_context(tc.tile_pool(name="persist", bufs=1))
        scp = ctx.enter_context(tc.tile_pool(name="scp", bufs=2, space="PSUM"))
        fillp = ctx.enter_context(tc.tile_pool(name="fillp", bufs=2, space="PSUM"))
        accp = ctx.enter_context(tc.tile_pool(name="accp", bufs=1, space="PSUM"))
        ptp = ctx.enter_context(tc.tile_pool(name="pts", bufs=1))
        srp = ctx.enter_context(tc.tile_pool(name="srp", bufs=2))
        ph3 = ctx.enter_context(tc.tile_pool(name="ph3", bufs=4))

        # ---- persistent tiles ----
        xT = persist.tile([128, NK, NT, 512], BF16, tag="xT")
        wq_sb = persist.tile([128, NK, DG], BF16, tag="wq_sb")
        wk_sb = persist.tile([128, NK, DG], BF16, tag="wk_sb")
        wv_sb = persist.tile([128, NK, DG], BF16, tag="wv_sb")
        wo_sb = persist.tile([128, 2, C], BF16, tag="wo_sb")
        keep_sb = persist.tile([128, T], BF16, tag="keep")
        qT = [persist.tile([128, T], BF16, tag=f"qT{i}", name=f"qT{i}") for i in range(2)]
        kT = [persist.tile([128, T], BF16, tag=f"kT{i}", name=f"kT{i}") for i in range(2)]
        vsbA = persist.tile([128, NTT, HPG, KH + 1], BF16, tag="vsbA")
        aT2 = [persist.tile([128, T], BF16, tag=f"aT2{i}", name=f"aT2{i}")
               for i in range(2)]
        a_hi = [persist.tile([64, T], BF16, tag=f"a_hi{i}", name=f"a_hi{i}")
                for i in range(2)]
        # bf16 rank-1 broadcast operands: f32r matmuls pay a serial 4-byte
        # weight load (no LDWEIGHTS support); bf16 runs at 1 cy/row with a
        # hideable split weight load.  bf16 rounding of the softmax
        # denominator adds <0.5% error, well inside the budget.
        ones_sb = persist.tile([65, 64], BF16, tag="ones")

        # Input DMAs, ordered so the t-block-0 projections can start early.
        def xt_blk(tb):
            return xt[tb * C:(tb + 1) * C, :].rearrange("(k p) t -> p k t", k=NK)

        nc.sync.dma_start(out=wk_sb, in_=wk.rearrange("(k p) d -> p k d", k=NK))
        nc.sync.dma_start(
            out=xT[:, 0:4, 0, :],
            in_=xt[0:512, :].rearrange("(k p) t -> p k t", k=4))
        nc.sync.dma_start(
            out=xT[:, 4:8, 0, :],
            in_=xt[512:1024, :].rearrange("(k p) t -> p k t", k=4))
        nc.sync.dma_start(out=keep_sb, in_=keep[:, :])
        nc.sync.dma_start(out=wq_sb, in_=wq.rearrange("(k p) d -> p k d", k=NK))
        nc.sync.dma_start(out=wv_sb, in_=wv.rearrange("(k p) d -> p k d", k=NK))
        nc.sync.dma_start(out=xT[:, :, 1, :], in_=xt_blk(1))
        nc.sync.dma_start(out=wo_sb, in_=wo.rearrange("(h p) c -> p h c", h=2))
        nc.sync.dma_start(out=xT[:, :, 2, :], in_=xt_blk(2))
        nc.sync.dma_start(out=xT[:, :, 3, :], in_=xt_blk(3))
        # ones column of the PV stationary (softmax denominator trick)
        nc.vector.memset(vsbA[:, :, :, KH:KH + 1], 1.0)
        nc.vector.memset(ones_sb, 1.0)

        # PE warm-up: ~5us of dummy matmuls on memset data, executed in the
        # shadow of the input DMAs.  Engages the HAM 8/8 clock (2.4 GHz)
        # deterministically before the first real matmul; without this the
        # p-state promotion races the fragmented early DMA-gated stream and
        # the whole kernel can run at 2.0 GHz.
        warm_w = persist.tile([128, 128], BF16, tag="warmw")
        warm_x = persist.tile([128, 512], BF16, tag="warmx")
        nc.vector.memset(warm_w, 0.0)
        nc.vector.memset(warm_x, 0.0)
        wps = scp.tile([128, 1024], F32, tag="sc", name="warmps")
        for _ in range(24):
            nc.tensor.matmul(wps[:, 0:512], warm_w, warm_x,
                             start=True, stop=True)

        # ---- fill units: generators that emit one instruction per step ----
        def qk_unit(dst, w_sb, tb, m):
            ps = fillp.tile([128, 512], F32, tag="f", name="psqk")
            for k in range(NK):
                nc.tensor.matmul(
                    ps, w_sb[:, k, m * 128:(m + 1) * 128], xT[:, k, tb, :],
                    start=(k == 0), stop=(k == NK - 1))
                yield
            nc.vector.tensor_copy(out=dst[m][:, tb * 512:(tb + 1) * 512], in_=ps)
            yield

        def v_unit(tt):
            tb, j = tt // 4, tt % 4
            ps = fillp.tile([128, 256], F32, tag="f", name="psv")
            for k in range(NK):
                nc.tensor.matmul(
                    ps, xT[:, k, tb, j * 128:(j + 1) * 128], wv_sb[:, k, :],
                    start=(k == 0), stop=(k == NK - 1))
                yield
            nc.vector.tensor_copy(
                out=vsbA[:, tt, :, 0:KH],
                in_=ps[:].rearrange("p (h d) -> p h d", h=HPG))
            yield

        def outproj_unit(tt, drain_act=False):
            # drain_act: in the tail the Scalar engine is idle (no more
            # EXPs) while DVE still runs the final norms — drain there.
            yt = ph3.tile([128, C], BF16, tag="ysb", name="yt")
            for no in range(2):
                ps = fillp.tile([128, 512], F32, tag="f", name="psy")
                for hp in range(2):
                    nc.tensor.matmul(
                        ps, aT2[hp][:, tt * 128:(tt + 1) * 128],
                        wo_sb[:, hp, no * 512:(no + 1) * 512],
                        start=(hp == 0), stop=(hp == 1))
                    yield
                if drain_act:
                    nc.scalar.copy(out=yt[:, no * 512:(no + 1) * 512], in_=ps)
                else:
                    nc.vector.tensor_copy(
                        out=yt[:, no * 512:(no + 1) * 512], in_=ps)
                yield
            nc.sync.dma_start(out=y[tt * 128:(tt + 1) * 128, :], in_=yt)
            yield

        def pump(fills, n):
            emitted = 0
            while emitted < n and fills:
                try:
                    next(fills[0])
                    emitted += 1
                except StopIteration:
                    fills.popleft()

        # ---- attention emission ----
        def emit_sc_kt(u, kt):
            qj, hp = u["qj"], u["hp"]
            off = 128 * (kt - 4 * qj) if kt >= 4 * qj else 0
            sc = scp.tile([128, 1024], F32, tag="sc", name="sc")
            for par in range(2):
                nc.tensor.matmul(
                    sc[:, par * 512 + off:(par + 1) * 512],
                    kT[hp][64 * par:64 * par + 64, kt * 128:(kt + 1) * 128],
                    qT[hp][64 * par:64 * par + 64,
                           qj * 512 + off:(qj + 1) * 512],
                    start=True, stop=True,
                )
            pt = ptp.tile([128, 1024], BF16, tag="pt", bufs=33, name="pt")
            if off == 0:
                # Flat contiguous AP: one row per partition on the Scalar
                # engine instead of two (saves per-row overhead on the
                # engine that paces the deep-qj units).
                nc.scalar.activation(out=pt[:, 0:1024], in_=sc[:, 0:1024],
                                     func=EXP, scale=0.125)
            else:
                nc.scalar.activation(
                    out=pt[:].rearrange("p (g c) -> p g c", g=2)[:, :, off:512],
                    in_=sc[:].rearrange("p (g c) -> p g c", g=2)[:, :, off:512],
                    func=EXP, scale=0.125)
            if kt >= 4 * qj:
                for par in range(2):
                    nc.vector.tensor_mul(
                        pt[:, par * 512 + off:par * 512 + off + 128],
                        pt[:, par * 512 + off:par * 512 + off + 128],
                        keep_sb[:, kt * 128:(kt + 1) * 128],
                    )
            u["pts"].append((pt, off))

        def emit_pv_kt(u, kt):
            hp, kmax = u["hp"], u["kmax"]
            if kt == 0:
                u["acc"] = accp.tile([65, 1024], F32, tag="acc", name="acc")
            pt, off = u["pts"][kt]
            for par in range(2):
                nc.tensor.matmul(
                    u["acc"][:, par * 512 + off:(par + 1) * 512],
                    vsbA[:, kt, 2 * hp + par, 0:KH + 1],
                    pt[:, par * 512 + off:(par + 1) * 512],
                    start=(kt == 0), stop=(kt == kmax - 1),
                )
            if kt == kmax - 1:
                u["pts"] = None

        def emit_pv_next(u, count):
            n = u.get("pv_next", 0)
            stop = min(n + count, u["kmax"])
            for kt in range(n, stop):
                emit_pv_kt(u, kt)
            u["pv_next"] = stop

        def emit_norm(u, per_tt=False):
            qj, hp, acc = u["qj"], u["hp"], u["acc"]
            rec_bc = srp.tile([64, 1024], F32, tag="rbc", name="rbc")
            # S row -> SBUF, rank-1 PE matmul broadcasts it across 64
            # partitions, then a fast reciprocal on DVE.
            srow = srp.tile([65, 1024], BF16, tag="srow", name="srow")
            rbcp = scp.tile([64, 1024], F32, tag="sc", name="rbcp")
            # per_tt: for the last unit, pipeline the whole chain in two
            # 256-column halves and normalize 128-t sub-blocks separately
            # so each output-projection tile unblocks as soon as its own
            # sub-block (and its a_hi DMA) lands.
            if per_tt:
                for ch in range(2):
                    lo, hi = ch * 256, (ch + 1) * 256
                    nc.vector.tensor_copy(
                        out=srow[64:65, :].rearrange(
                            "p (g c) -> p g c", g=2)[:, :, lo:hi],
                        in_=acc[64:65, :].rearrange(
                            "p (g c) -> p g c", g=2)[:, :, lo:hi])
                    for par in range(2):
                        nc.tensor.matmul(
                            rbcp[:, par * 512 + lo:par * 512 + hi],
                            ones_sb[64:65, :],
                            srow[64:65, par * 512 + lo:par * 512 + hi],
                            start=True, stop=True)
                        nc.vector.reciprocal_approx_fast(
                            out=rec_bc[:, par * 512 + lo:par * 512 + hi],
                            in_=rbcp[:, par * 512 + lo:par * 512 + hi])
                    for st in (2 * ch, 2 * ch + 1):
                        slo, shi = st * 128, (st + 1) * 128
                        nc.vector.tensor_mul(
                            aT2[hp][0:64, qj * 512 + slo:qj * 512 + shi],
                            acc[0:64, slo:shi],
                            rec_bc[:, slo:shi],
                        )
                        nc.vector.tensor_mul(
                            a_hi[hp][:, qj * 512 + slo:qj * 512 + shi],
                            acc[0:64, 512 + slo:512 + shi],
                            rec_bc[:, 512 + slo:512 + shi],
                        )
                        nc.sync.dma_start(
                            out=aT2[hp][64:128, qj * 512 + slo:qj * 512 + shi],
                            in_=a_hi[hp][:, qj * 512 + slo:qj * 512 + shi],
                        )
                u["acc"] = None
                return
            nc.vector.tensor_copy(out=srow[64:65, :], in_=acc[64:65, :])
            for par in range(2):
                nc.tensor.matmul(
                    rbcp[:, par * 512:(par + 1) * 512],
                    ones_sb[64:65, :],
                    srow[64:65, par * 512:(par + 1) * 512],
                    start=True, stop=True)
            nc.vector.reciprocal_approx_fast(out=rec_bc, in_=rbcp)
            nc.vector.tensor_mul(
                aT2[hp][0:64, qj * 512:(qj + 1) * 512],
                acc[0:64, 0:512],
                rec_bc[:, 0:512],
            )
            nc.vector.tensor_mul(
                a_hi[hp][:, qj * 512:(qj + 1) * 512],
                acc[0:64, 512:1024],
                rec_bc[:, 512:1024],
            )
            nc.sync.dma_start(
                out=aT2[hp][64:128, qj * 512:(qj + 1) * 512],
                in_=a_hi[hp][:, qj * 512:(qj + 1) * 512],
            )
            u["acc"] = None

        # Prologue: Q/K projections for t-block 0 (K first: score stationary).
        for gen in [qk_unit(kT, wk_sb, 0, 0), qk_unit(kT, wk_sb, 0, 1),
                    qk_unit(qT, wq_sb, 0, 0), qk_unit(qT, wq_sb, 0, 1)]:
            for _ in gen:
                pass

        # Fill units per attention stage (qj): V of this stage's t-block,
        # Q/K projections of the next t-block, output projections of
        # already-normalized t-blocks.  Later output projections are held
        # back to cover the EXP-bound deep-qj units and the final norm
        # latency in the tail.
        STAGE_FILLS = {
            0: [("v", 0), ("v", 1), ("v", 2), ("v", 3),
                ("k", 1, 0), ("k", 1, 1), ("q", 1, 0), ("q", 1, 1)],
            1: [("v", 4), ("v", 5), ("v", 6), ("v", 7),
                ("k", 2, 0), ("k", 2, 1), ("q", 2, 0), ("q", 2, 1),
                ("o", 0), ("o", 1)],
            2: [("v", 8), ("v", 9), ("v", 10), ("v", 11),
                ("k", 3, 0), ("k", 3, 1), ("q", 3, 0), ("q", 3, 1),
                ("o", 2), ("o", 3)],
            3: [("v", 12), ("v", 13), ("v", 14), ("v", 15),
                ("o", 4), ("o", 5), ("o", 6), ("o", 7),
                ("o", 8), ("o", 9)],
        }
        BUDGET = [9, 6, 4, 4]

        def make_unit(spec):
            if spec[0] == "v":
                return v_unit(spec[1])
            if spec[0] == "k":
                return qk_unit(kT, wk_sb, spec[1], spec[2])
            if spec[0] == "q":
                return qk_unit(qT, wq_sb, spec[1], spec[2])
            return outproj_unit(spec[1])

        pv_pend = None
        for qj in range(NT):
            fills = deque(make_unit(s) for s in STAGE_FILLS[qj])
            for hp in range(2):
                u = {"qj": qj, "hp": hp, "kmax": 4 * qj + 4, "pts": []}
                # Score pairs in batches of 2 kt (64x128 row-tile mode),
                # then PV + fills (128x128 mode): the PE pays a ~100ns
                # drain per tiling-mode switch, so batching halves it.
                # The previous unit's pt tiles are all complete, so its PV
                # runs at 2 kt per slot and its norm lands mid-unit --
                # releasing the single PSUM acc buffer well before this
                # unit's own PV needs it.
                pv_rate = 6 if qj == 3 else 4
                for kt in range(u["kmax"]):
                    emit_sc_kt(u, kt)
                    # Last unit: start its own PV with a short trail once
                    # the acc buffer has been freed by the mid-unit norm,
                    # shortening the dense PV tail after the loop.
                    if qj == 3 and hp == 1 and kt >= 9:
                        emit_pv_next(u, 1)
                    if kt % 2 == 1:
                        if pv_pend is not None:
                            emit_pv_next(pv_pend, pv_rate)
                            if pv_pend["pv_next"] >= pv_pend["kmax"]:
                                emit_norm(pv_pend)
                                pv_pend = None
                        pump(fills, 2 * BUDGET[qj])
                if pv_pend is not None:
                    emit_norm(pv_pend)
                    pv_pend = None
                pv_pend = u
            # drain remaining fills of this stage
            pump(fills, 10000)

        # Tail: PV + norm of the last unit; the held-back output
        # projections cover the final norm latency.
        emit_pv_next(pv_pend, pv_pend["kmax"])
        emit_norm(pv_pend, per_tt=True)
        # op(10,11) depend only on stage-2 norms: they fill the PE while
        # the final norm chain runs on DVE.
        tail = deque([outproj_unit(tt) for tt in (10, 11, 12, 13, 14, 15)])
        pump(tail, 10000)

    _split_excess_waits(nc)
    nc.compile()
    if HOIST:
        _hoist_pair_ldweights(nc)
    return nc


def _split_excess_waits(nc):
    """Walrus caps most instructions at 1 sync wait. Peel excess waits off
    matmuls (and anything else over the cap) onto PE-engine wait-nops
    inserted immediately before the instruction."""
    for bb in nc.main_func.blocks:
        new_insts = []
        for inst in bb.instructions:
            si = inst.sync_info
            if (si is not None and si.on_wait and len(si.on_wait) > 1
                    and isinstance(inst, mybir.InstMatmult)):
                excess = list(si.on_wait[:-1])
                keep = [si.on_wait[-1]]
                for w in excess:
                    nop = mybir.InstNoOp(
                        name=nc.get_next_instruction_name(), ins=[], outs=[],
                        bass_nofuse=True)
                    nop.engine = inst.engine
                    nop.sync_info = mybir.SyncInfo(on_wait=[w], on_update=[])
                    nc.register_instruction(nop)
                    new_insts.append(nop)
                si.on_wait = keep
            new_insts.append(inst)
        bb.instructions[:] = new_insts


def _hoist_pair_ldweights(nc):
    """Reorder Ldw(T0) MM(T0) Ldw(T8) MM(T8) score-pair sequences into
    Ldw(T0) Ldw(T8) MM(T0) MM(T8) so the two 64x128 row tiles of the PE
    array have their weights resident before either matmul issues and the
    pair executes concurrently.  Safe when Ldw(T8) carries no semaphore
    updates (engine-sem counts seen by consumers are unchanged)."""
    for bb in nc.main_func.blocks:
        ins = bb.instructions
        out = []
        i = 0
        n = len(ins)
        while i < n:
            if (i + 3 < n
                    and isinstance(ins[i], mybir.InstLdweights)
                    and isinstance(ins[i + 1], mybir.InstMatmult)
                    and isinstance(ins[i + 2], mybir.InstLdweights)
                    and isinstance(ins[i + 3], mybir.InstMatmult)
                    and ins[i + 1].tile_size == (64, 128)
                    and ins[i + 3].tile_size == (64, 128)
                    and ins[i + 1].tile_position == (0, 0)
                    and ins[i + 3].tile_position == (64, 0)
                    and not (ins[i + 2].sync_info is not None
                             and ins[i + 2].sync_info.on_update)):
                out += [ins[i], ins[i + 2], ins[i + 1], ins[i + 3]]
                i += 4
            else:
                out.append(ins[i])
                i += 1
        bb.instructions[:] = out


def _host_prep(x, Wq, Wkv, Wout, mask):
    import ml_dtypes
    BF = ml_dtypes.bfloat16

    x = np.asarray(x, dtype=np.float32)
    Wq = np.asarray(Wq, dtype=np.float32)
    Wkv = np.asarray(Wkv, dtype=np.float32)
    Wout = np.asarray(Wout, dtype=np.float32)
    mask = np.asarray(mask)

    # x[b].T split into 4 contiguous t-blocks: [4*C, 512]
    xTb = []
    for b in range(B):
        xt = np.ascontiguousarray(x[b].T)                    # [C, T]
        xt = np.ascontiguousarray(
            xt.reshape(C, NT, 512).transpose(1, 0, 2)).astype(BF)
        xTb.append(xt.reshape(NT * C, 512))
    keep = np.empty((128, T), dtype=np.float32)
    for i in range(T // 128):
        blk = mask[128 * i:128 * (i + 1), 128 * i:128 * (i + 1)]
        keep[:, 128 * i:128 * (i + 1)] = (~blk).T.astype(np.float32)
    keep = keep.astype(BF)

    in_maps = []
    for core in range(NCORES):
        b, g = core // G, core % G
        sl = slice(DG * g, DG * (g + 1))
        in_maps.append({
            "xt": xTb[b],
            "wq": np.ascontiguousarray(Wq[sl, :].T).astype(BF),
            "wk": np.ascontiguousarray(Wkv[sl, :].T).astype(BF),
            "wv": np.ascontiguousarray(Wkv[C + DG * g:C + DG * (g + 1), :].T).astype(BF),
            "wo": np.ascontiguousarray(Wout[:, sl].T).astype(BF),
            "keep": keep,
        })
    return in_maps


def _install_ntff_hook():
    import types
    import antenv
    if getattr(antenv, "axon_hooks", None) is not None:
        return
    ah = types.ModuleType("antenv.axon_hooks")
    ah._hook = None
    ah.set_axon_ntff_profile_hook = lambda h: setattr(ah, "_hook", h)
    ah.get_axon_ntff_profile_hook = lambda: ah._hook
    sys.modules["antenv.axon_hooks"] = ah
    antenv.axon_hooks = ah
    if "/root/.axon_site" not in sys.path:
        sys.path.insert(0, "/root/.axon_site")
    from trn_agent_boot.trn_boot import _ntff_profile_via_ctypes
    ah.set_axon_ntff_profile_hook(_ntff_profile_via_ctypes("/opt/axon/libaxon_pjrt.so"))


def _run(inputs, trace=False):
    global _cached_nc
    from concourse.bass_utils import run_bass_kernel_spmd
    if trace:
        _install_ntff_hook()
    if _cached_nc is None:
        _cached_nc = build_nc()
    in_maps = _host_prep(**inputs)
    res = run_bass_kernel_spmd(_cached_nc, in_maps, list(range(NCORES)), trace=trace)
    parts = [np.asarray(res.results[c]["y"], dtype=np.float32)
             for c in range(NCORES)]
    out = np.stack([
        parts[0] + parts[1] + parts[2] + parts[3],
        parts[4] + parts[5] + parts[6] + parts[7],
    ]).astype(np.float32)
    return out, res


def kernel(x, Wq, Wkv, Wout, mask):
    out, _ = _run(dict(x=x, Wq=Wq, Wkv=Wkv, Wout=Wout, mask=mask))
    return out
